# revision 41
# baseline (speedup 1.0000x reference)
"""CrossAttention Trainium2 kernel, v2.

Problem (hardcoded): B=8, T=256, S=4096, E=512, KV=768, H=8, D=64.
Sharding: data-parallel over B — one batch per NeuronCore (8 cores).

Key ideas vs v1:
  * Key compaction: ~50% of keys are masked. Host gathers kept keys per
    batch and pads to S_C = ceil(max_kept/128)*128 (2176 for the bench
    seed). All S-proportional work (KV proj, scores, exp, PV) shrinks
    by ~47%. Pad rows have zero context columns -> k=0 -> score 0 ->
    exp 1, but vp rows and the ones-column are 0 so they contribute
    nothing to numerator or denominator (m01 carries kept/pad only).
  * K-projection runs in fp8-e4m3 DoubleRow (contraction 256/pass):
    Wk is rescaled x64 into e4m3's normal range and 2^-6 is folded into
    Wq. Measured end-to-end rel err 1.37e-2 (gate 2e-2), 10us faster.
  * Group-pipelined emission: ctx arrives in groups of 4 s-chunks
    (tapered to 2,2,1 at the end); K/V projection of group g is
    interleaved instruction-by-instruction with attention of group g-1
    so the PE never waits on the scalar-engine exp.
  * Scores keep PE row-group-0 (head-even) and row-group-64 (head-odd)
    outputs in SEPARATE psum banks — mixing them in one bank makes the
    runtime reject the NEFF (PE quadrants are wired to PSUM halves).
  * PV accumulates per group in [65,256] psum tiles (row 64 = softmax
    denominator via a ones-column in vp), folded into SBUF accumulators
    by the DVE.
  * Per-head-pair normalization starts as soon as that pair's last fold
    lands: denominator row is DMA-spread across 128 partitions for a
    fast reciprocal, gathered back, broadcast down 64 rows with a K=1
    matmul; tiny keepalive matmuls tied to the chain keep the PE's HAM
    activity window warm. The out projection contracts in 64-row chunks
    so each head pair feeds it immediately.
"""

import os
import sys

sys.path.insert(0, "/opt/trn_rl_repo")

# Debug truncation: 1=proj, 2=+scores, 3=+exp, 4=+pv, 5=+norm, 6=full (default)
V2_STAGE = int(os.environ.get("V2_STAGE", "6"))
V2_NOILV = os.environ.get("V2_NOILV", "") == "1"  # emit proj before attn (no interleave)

import numpy as np
import ml_dtypes
from contextlib import ExitStack

import concourse.bass as bass
import concourse.bacc as bacc
import concourse.tile as tile
from concourse import mybir
from concourse import bass_utils

BF16 = mybir.dt.bfloat16
F32 = mybir.dt.float32
F8E4 = mybir.dt.float8e4
NPBF16 = ml_dtypes.bfloat16
NPF8 = ml_dtypes.float8_e4m3fn

B, T, S, E, KV, H, D = 8, 256, 4096, 512, 768, 8, 64
NC_CORES = 8
GROUP_SC = 4  # s-chunks per pipeline group


def _build_program(n_sc):
    S_C = n_sc * 128
    sizes = []
    rem = n_sc
    if rem > 7:
        sizes.append(2)  # small first group: projections start sooner
        rem -= 2
    while rem > 5:
        sizes.append(GROUP_SC)
        rem -= GROUP_SC
    while rem > 0:
        n = 2 if rem > 2 else rem
        sizes.append(n)
        rem -= n
    groups = []
    sc0 = 0
    for n in sizes:
        groups.append((sc0, n))
        sc0 += n
    G = len(groups)

    nc = bacc.Bacc("TRN2", target_bir_lowering=False, debug=False)

    ctxb_d = nc.dram_tensor("ctxb", [128, 6 * S_C], BF16, kind="ExternalInput").ap()
    ctx8_d = nc.dram_tensor("ctx8", [128, 6 * S_C], F8E4, kind="ExternalInput").ap()
    wkvk8_d = nc.dram_tensor("wkvk8", [128, 3 * 1024], F8E4, kind="ExternalInput").ap()
    x_d = nc.dram_tensor("xr", [128, 4 * T], BF16, kind="ExternalInput").ap()
    m01_d = nc.dram_tensor("m01", [128, n_sc], F32, kind="ExternalInput").ap()
    wq_d = nc.dram_tensor("wqr", [128, 4 * 512], BF16, kind="ExternalInput").ap()
    wkvv_d = nc.dram_tensor("wkvv", [128, 6 * 512], BF16, kind="ExternalInput").ap()
    wo_d = nc.dram_tensor("wo64", [64, 8 * 512], BF16, kind="ExternalInput").ap()
    bo_d = nc.dram_tensor("bo_r", [128, 4], F32, kind="ExternalInput").ap()
    id128_d = nc.dram_tensor("id128", [128, 128], BF16, kind="ExternalInput").ap()
    outT_d = nc.dram_tensor("outT", [4, 128, T], F32, kind="ExternalOutput").ap()

    with tile.TileContext(nc) as tc, ExitStack() as ctx:
        const = ctx.enter_context(tc.tile_pool(name="const", bufs=1))
        work = ctx.enter_context(tc.tile_pool(name="work", bufs=2))
        psum = ctx.enter_context(tc.tile_pool(name="psum", bufs=1, space="PSUM"))

        # ---- static SBUF tensors ------------------------------------------
        ctxb_t = const.tile([128, 6 * S_C], BF16, tag="ctxb", name="ctxb")
        ctx8_t = const.tile([128, 6 * S_C], F8E4, tag="ctx8", name="ctx8")
        wkvk8_t = const.tile([128, 3 * 1024], F8E4, tag="wkvk8", name="wkvk8")
        x_t = const.tile([128, 4 * T], BF16, tag="x", name="x")
        wq_t = const.tile([128, 4 * 512], BF16, tag="wq", name="wq")
        wkvv_t = const.tile([128, 6 * 512], BF16, tag="wkvv", name="wkvv")
        wo_t = const.tile([64, 8 * 512], BF16, tag="wo", name="wo")
        bo_t = const.tile([128, 4], F32, tag="bo", name="bo")
        m01_t = const.tile([128, n_sc], F32, tag="m01", name="m01")
        kt_t = [
            const.tile([128, S_C], BF16, tag=f"kt{kc}", name=f"kt{kc}")
            for kc in range(4)
        ]
        vp_t = [
            const.tile([128, 8 * 65], BF16, tag=f"vp{sc}", name=f"vp{sc}")
            for sc in range(n_sc)
        ]
        qt_t = [
            const.tile([128, T], BF16, tag=f"qt{qc}", name=f"qt{qc}") for qc in range(4)
        ]
        otE_t = [
            const.tile([64, T], BF16, tag=f"otE{kc}", name=f"otE{kc}")
            for kc in range(4)
        ]
        otO_t = [
            const.tile([64, T], BF16, tag=f"otO{kc}", name=f"otO{kc}")
            for kc in range(4)
        ]
        ones8_t = const.tile([128, 8], BF16, tag="ones8", name="ones8")
        ones64_t = const.tile([1, 64], BF16, tag="ones64", name="ones64")
        id65f_t = const.tile([65, 1], F32, tag="id65f", name="id65f")
        id128_t = const.tile([128, 128], BF16, tag="id128", name="id128")
        recb_t = const.tile([128, 16], BF16, tag="recb", name="recb")
        rechp_t = const.tile([1, 4 * 512], BF16, tag="rechp", name="rechp")
        dummy_t = const.tile([1, 64], BF16, tag="dummy", name="dummy")

        # SBUF accumulators for PV (head pair packed along free; row 64 = denom)
        pvacc_t = [
            const.tile([65, 512], F32, tag=f"pvacc{kc}", name=f"pvacc{kc}")
            for kc in range(4)
        ]

        # ---- tiny init + ACT table warm -----------------------------------
        nc.vector.memset(ones8_t[:], 1.0)
        nc.vector.memset(id65f_t[:], 1.0)
        nc.vector.memset(ones64_t[:], 1.0)
        nc.scalar.activation(
            dummy_t[:], ones64_t[:], mybir.ActivationFunctionType.Exp
        )

        # ---- DMA dispatches (two hw queues; priority order) ---------------
        x_v = x_t[:].rearrange("p (e t) -> p e t", e=4)
        xd_v = x_d.rearrange("p (e t) -> p e t", e=4)
        wq_v = wq_t[:].rearrange("p (e m) -> p e m", e=4)
        wqd_v = wq_d.rearrange("p (e m) -> p e m", e=4)
        for ec in range(4):
            nc.sync.dma_start(x_v[:, ec], xd_v[:, ec])
            nc.gpsimd.dma_start(wq_v[:, ec], wqd_v[:, ec])
        nc.sync.dma_start(wkvk8_t[:], wkvk8_d)
        ctxb_v = ctxb_t[:].rearrange("p (c s) -> p c s", c=6)
        ctxd_v = ctxb_d.rearrange("p (c s) -> p c s", c=6)
        ctx8_v = ctx8_t[:].rearrange("p (c s) -> p c s", c=6)
        ctx8d_v = ctx8_d.rearrange("p (c s) -> p c s", c=6)
        wkvv_v = wkvv_t[:].rearrange("p (c m) -> p c m", c=6)
        wkvvd_v = wkvv_d.rearrange("p (c m) -> p c m", c=6)
        g0_, gn_ = groups[0]
        cols0 = slice(g0_ * 128, g0_ * 128 + gn_ * 128)
        nc.sync.dma_start(ctx8_v[:, :, cols0], ctx8d_v[:, :, cols0])
        # interleave wkvv and ctx-bf16 group-0 chunks so V-proj can start
        # accumulating as soon as the first c-chunks land
        for c in range(6):
            q = nc.sync if c % 2 == 0 else nc.gpsimd
            q.dma_start(wkvv_v[:, c, :], wkvvd_v[:, c, :])
            q2 = nc.gpsimd if c % 2 == 0 else nc.sync
            q2.dma_start(ctxb_v[:, c, cols0], ctxd_v[:, c, cols0])
        nc.gpsimd.dma_start(m01_t[:], m01_d)
        for gi, (g0, gn) in enumerate(groups[1:], 1):
            cols = slice(g0 * 128, g0 * 128 + gn * 128)
            q = nc.sync if gi % 2 == 0 else nc.gpsimd
            q.dma_start(ctx8_v[:, :, cols], ctx8d_v[:, :, cols])
            for c in range(6):
                q.dma_start(ctxb_v[:, c, cols], ctxd_v[:, c, cols])
        nc.gpsimd.dma_start(wo_t[:], wo_d)
        nc.gpsimd.dma_start(bo_t[:], bo_d)
        nc.gpsimd.dma_start(id128_t[:], id128_d)

        def emit_qproj():
            for qc in range(4):
                ps = psum.tile([128, T], F32, tag="w", bufs=2, name="qps")
                for ec in range(4):
                    nc.tensor.matmul(
                        ps[:],
                        lhsT=wq_t[
                            :, ec * 512 + qc * 128 : ec * 512 + (qc + 1) * 128
                        ],
                        rhs=x_t[:, ec * T : (ec + 1) * T],
                        start=(ec == 0),
                        stop=(ec == 3),
                    )
                nc.scalar.copy(qt_t[qc][:], ps[:])

        # ---- pipeline: proj(g) interleaved with attn(g-1) -----------------
        def proj_ops(gi):
            g0, gn = groups[gi]
            cols = slice(g0 * 128, g0 * 128 + gn * 128)
            ops = []
            for kc in range(4):

                def k_proj(kc=kc, cols=cols, gn=gn, g0=g0, gi=gi):
                    ps = psum.tile([128, 512], F32, tag="w", bufs=2, name="kps")
                    w8 = wkvk8_t[:].rearrange("p (d j m) -> p d j m", d=3, j=2)
                    for dc in range(3):
                        nc.tensor.matmul(
                            ps[:, 0 : gn * 128],
                            lhsT=w8[:, dc, :, kc * 128 : (kc + 1) * 128],
                            rhs=ctx8_v[:, 2 * dc : 2 * dc + 2, cols],
                            start=(dc == 0),
                            stop=(dc == 2),
                            perf_mode=mybir.MatmulPerfMode.DoubleRow,
                        )
                    if gi % 2 == 0:
                        nc.scalar.copy(kt_t[kc][:, cols], ps[:, 0 : gn * 128])
                    else:
                        nc.vector.tensor_copy(kt_t[kc][:, cols], ps[:, 0 : gn * 128])

                ops.append(k_proj)
            for sc in range(g0, g0 + gn):

                def v_proj(sc=sc):
                    ps = psum.tile([128, 512], F32, tag="w", bufs=2, name="vps")
                    for c in range(6):
                        nc.tensor.matmul(
                            ps[:],
                            lhsT=ctxb_v[:, c, sc * 128 : (sc + 1) * 128],
                            rhs=wkvv_t[:, c * 512 : (c + 1) * 512],
                            start=(c == 0),
                            stop=(c == 5),
                        )
                    dst = vp_t[sc][:].rearrange("p (h e) -> p h e", e=65)
                    nc.vector.tensor_copy(
                        dst[:, :, 0:64], ps[:].rearrange("p (h d) -> p h d", d=64)
                    )
                    nc.vector.tensor_scalar_mul(
                        dst[:, :, 64:65],
                        ones8_t[:].rearrange("p (h o) -> p h o", o=1),
                        m01_t[:, sc : sc + 1],
                    )

                ops.append(v_proj)
            return ops

        def attn_units(gi):
            g0, gn = groups[gi]
            pairs = []
            p0 = g0
            while p0 < g0 + gn:
                np_ = min(2, g0 + gn - p0)
                pairs.append((p0, np_))
                p0 += np_
            return [(kc, p0, np_) for kc in range(4) for (p0, np_) in pairs]

        pend_pv = []  # [(kc, p0, np_, eE, eO)]
        n_done = [0] * 4  # PV s-chunks accumulated per kc
        pvq = [None] * 4  # current per-group psum pair for each kc
        fold_done = [False] * 4  # pvacc initialized yet?

        def sc_group(sc):
            for g0, gn in groups:
                if g0 <= sc < g0 + gn:
                    return g0, gn
            raise AssertionError

        def emit_scores(kc, p0, np_):
            if V2_STAGE < 2:
                return
            # separate psum tiles per head: row-group-0 and row-group-64
            # outputs must not share a psum bank
            psE = psum.tile([128, 512], F32, tag="sc", bufs=2, name="psE")
            psO = psum.tile([128, 512], F32, tag="sc", bufs=2, name="psO")
            for i in range(np_):
                sc = p0 + i
                nc.tensor.matmul(
                    psE[:, i * T : (i + 1) * T],
                    lhsT=kt_t[kc][0:64, sc * 128 : (sc + 1) * 128],
                    rhs=qt_t[kc][0:64, :],
                    start=True,
                    stop=True,
                )
            for i in range(np_):
                sc = p0 + i
                nc.tensor.matmul(
                    psO[:, i * T : (i + 1) * T],
                    lhsT=kt_t[kc][64:128, sc * 128 : (sc + 1) * 128],
                    rhs=qt_t[kc][64:128, :],
                    start=True,
                    stop=True,
                )
            if V2_STAGE < 3:
                return
            eE = work.tile([128, 512], BF16, tag="e", bufs=6, name="eE")
            eO = work.tile([128, 512], BF16, tag="e", bufs=6, name="eO")
            nc.scalar.activation(
                eE[:, 0 : np_ * T], psE[:, 0 : np_ * T],
                mybir.ActivationFunctionType.Exp,
            )
            nc.scalar.activation(
                eO[:, 0 : np_ * T], psO[:, 0 : np_ * T],
                mybir.ActivationFunctionType.Exp,
            )
            if V2_STAGE < 4:
                return
            pend_pv.append((kc, p0, np_, eE, eO))

        def emit_pv():
            kc, p0, np_, eE, eO = pend_pv.pop(0)
            g0, gn = sc_group(p0)
            if p0 == g0:
                pvq[kc] = [
                    psum.tile([65, T], F32, tag="pv", bufs=4, name=f"pvq{hi}")
                    for hi in range(2)
                ]
            for i in range(np_):
                sc = p0 + i
                for hi, e in ((0, eE), (1, eO)):
                    h = 2 * kc + hi
                    nc.tensor.matmul(
                        pvq[kc][hi][:],
                        lhsT=vp_t[sc][:, h * 65 : h * 65 + 65],
                        rhs=e[:, i * T : (i + 1) * T],
                        start=(sc == g0),
                        stop=(sc == g0 + gn - 1),
                    )
            if p0 + np_ == g0 + gn:
                # fold the group's PV into the SBUF accumulator (DVE);
                # first fold in processing order initializes it
                for hi in range(2):
                    dst = pvacc_t[kc][:, hi * T : (hi + 1) * T]
                    if not fold_done[kc]:
                        nc.vector.tensor_copy(dst, pvq[kc][hi][:])
                    else:
                        nc.vector.tensor_add(dst, dst, pvq[kc][hi][:])
                fold_done[kc] = True
            n_done[kc] += np_
            if n_done[kc] == n_sc and V2_STAGE >= 5:
                emit_norm(kc)

        def emit_norm(kc):
            # denominators live in pvacc row 64 (cols 0:256 headE, 256:512 headO).
            # Spread the 512 values across 128 partitions (t = p*4+j) so the
            # reciprocal uses 128 DVE lanes instead of 1, then gather back.
            # spread den across 128 partitions via PE transpose (no DMA hop:
            # engine-semaphore sync only, and the PE stays active)
            dsp = psum.tile([128, 512], F32, tag="sc", bufs=2, name="dsp")
            for c in range(4):
                nc.tensor.matmul(
                    dsp[:, c : c + 1],
                    lhsT=pvacc_t[kc][64:65, c * 128 : (c + 1) * 128],
                    rhs=id65f_t[64:65, :],
                    start=True,
                    stop=True,
                    is_transpose=True,
                )
            with nc.allow_low_precision(reason="softmax denom reciprocal in bf16"):
                nc.vector.reciprocal(
                    recb_t[:, kc * 4 : (kc + 1) * 4], dsp[:, 0:4]
                )
            # gather back to one [1,512] row via 4 transposes, then to SBUF
            rgp = psum.tile([1, 512], BF16, tag="sc", bufs=2, name="rgp")
            for j in range(4):
                nc.tensor.matmul(
                    rgp[0:1, j * 128 : (j + 1) * 128],
                    lhsT=recb_t[:, kc * 4 + j : kc * 4 + j + 1],
                    rhs=id128_t[:],
                    start=True,
                    stop=True,
                    is_transpose=True,
                )
            nc.vector.tensor_copy(
                rechp_t[0:1, kc * 512 : (kc + 1) * 512], rgp[0:1, :]
            )
            bc = psum.tile([128, 512], F32, tag="sc", bufs=2, name="bc")
            nc.tensor.matmul(
                bc[0:64, :],
                lhsT=ones64_t[:],
                rhs=rechp_t[0:1, kc * 512 : (kc + 1) * 512],
                start=True,
                stop=True,
            )
            nc.vector.tensor_mul(otE_t[kc][:], pvacc_t[kc][0:64, 0:T], bc[0:64, 0:T])
            nc.vector.tensor_mul(
                otO_t[kc][:], pvacc_t[kc][0:64, T : 2 * T], bc[0:64, T : 2 * T]
            )
            if V2_STAGE < 6:
                return
            # out projection contribution of this head pair (64-row chunks)
            for j, ot in ((2 * kc, otE_t[kc]), (2 * kc + 1, otO_t[kc])):
                for oi in range(4):
                    nc.tensor.matmul(
                        out_ps[oi // 2][:, (oi % 2) * T : (oi % 2 + 1) * T],
                        lhsT=wo_t[:, j * 512 + oi * 128 : j * 512 + (oi + 1) * 128],
                        rhs=ot[:],
                        start=(j == 0 and oi % 2 == 0),
                        stop=(j == 7 and oi % 2 == 1),
                    )

        # Reserve early-group attention units per kc and drain them in the
        # final phase: staggers the four pv completions so each norm chain
        # (den DMA -> recip -> gather -> bc -> muls -> outproj) overlaps real
        # PE/ACT work instead of stalling the PE (and re-throttling HAM).
        reserved = {kc: set() for kc in range(4)}

        def group_index(g0):
            for idx, (gg0, _) in enumerate(groups):
                if gg0 == g0:
                    return idx
            raise AssertionError

        out_ps = None
        emit_qproj()
        for gi in range(G + 1):
            units = attn_units(gi - 1) if gi >= 1 else []
            units = [
                (kc, p0, np_)
                for (kc, p0, np_) in units
                if group_index(sc_group(p0)[0]) not in reserved[kc]
            ]
            pops = proj_ops(gi) if gi < G else []
            if gi == G:
                # final phase: append reserved units ordered so each kc's
                # completion lands a few units after the previous kc's norm
                for kc in range(1, 4):
                    for g_idx in sorted(reserved[kc]):
                        g0r, gnr = groups[g_idx]
                        p0 = g0r
                        while p0 < g0r + gnr:
                            np_ = min(2, g0r + gnr - p0)
                            units.append((kc, p0, np_))
                            p0 += np_
                # allocate out-projection accumulators (w-ring is free now)
                out_ps = [
                    psum.tile([128, 512], F32, tag="w", bufs=2, name=f"ops{i}")
                    for i in range(2)
                ]
            if V2_STAGE < 2:
                units = []
            if not units:
                for op in pops:
                    op()
                continue
            # interleave proj ops between attention units
            if V2_NOILV:
                for op in pops:
                    op()
                pops = []
            k = 0
            for i, (kc, p0, np_) in enumerate(units):
                emit_scores(kc, p0, np_)
                while len(pend_pv) > 1:
                    emit_pv()
                k_to = (i + 1) * len(pops) // len(units) if pops else 0
                while k < k_to:
                    pops[k]()
                    k += 1
        while pend_pv:
            emit_pv()

        # ---- bias + store -------------------------------------------------
        for half in range(2):
            osb = work.tile([128, 512], F32, tag="osb", bufs=2, name="osb")
            if V2_STAGE >= 6:
                for eo2 in range(2):
                    eo = half * 2 + eo2
                    if half == 0:
                        nc.vector.tensor_scalar_add(
                            osb[:, eo2 * T : (eo2 + 1) * T],
                            out_ps[half][:, eo2 * T : (eo2 + 1) * T],
                            bo_t[:, eo : eo + 1],
                        )
                    else:
                        nc.scalar.add(
                            osb[:, eo2 * T : (eo2 + 1) * T],
                            out_ps[half][:, eo2 * T : (eo2 + 1) * T],
                            bo_t[:, eo : eo + 1],
                        )
            else:
                nc.vector.memset(osb[:], 0.0)
            for eo2 in range(2):
                q = nc.sync if eo2 == 0 else nc.gpsimd
                q.dma_start(
                    outT_d[2 * half + eo2], osb[:, eo2 * T : (eo2 + 1) * T]
                )

    nc.compile()
    return nc


_NC = {}


def _get_nc(n_sc):
    if n_sc not in _NC:
        _NC[n_sc] = _build_program(n_sc)
    return _NC[n_sc]


def _prep_in_maps(x, context, key_padding_mask, Wq, Wkv, Wo, bo):
    keep = ~np.asarray(key_padding_mask)
    kept = keep.sum(axis=1)
    n_sc = max(1, -(-int(kept.max()) // 128))
    S_C = n_sc * 128

    scale = np.float32(D**-0.5) * np.float32(2.0**-6)
    wqr = (
        (np.ascontiguousarray(Wq.T) * scale)
        .reshape(4, 128, 4 * 128)
        .transpose(1, 0, 2)
        .reshape(128, 4 * 512)
        .astype(NPBF16)
    )
    wkvT = np.ascontiguousarray(Wkv.T)  # [768, 1024]
    wkvk8 = (
        (wkvT[:, :512] * np.float32(64.0))
        .astype(NPF8)
        .reshape(3, 2, 128, 512)
        .transpose(2, 0, 1, 3)
        .reshape(128, 3 * 1024)
    )
    wkvv = (
        wkvT[:, 512:].reshape(6, 128, 512).transpose(1, 0, 2).reshape(128, 6 * 512)
    ).astype(NPBF16)
    wo64 = (
        np.ascontiguousarray(Wo.T)
        .reshape(8, 64, 512)
        .transpose(1, 0, 2)
        .reshape(64, 8 * 512)
    ).astype(NPBF16)
    bo_r = np.ascontiguousarray(bo.reshape(4, 128).T).astype(np.float32)
    id128 = np.eye(128, dtype=np.float32).astype(NPBF16)

    in_maps = []
    for b in range(B):
        idx = np.nonzero(keep[b])[0]
        k_b = len(idx)
        ctxT = np.zeros((KV, S_C), np.float32)
        ctxT[:, :k_b] = context[b][idx].T
        ctxr = ctxT.reshape(6, 128, S_C).transpose(1, 0, 2).reshape(128, 6 * S_C)
        ctxb = ctxr.astype(NPBF16)
        ctx8 = ctxr.astype(NPF8)
        xr = (
            np.ascontiguousarray(x[b].T)
            .reshape(4, 128, T)
            .transpose(1, 0, 2)
            .reshape(128, 4 * T)
        ).astype(NPBF16)
        m01 = np.zeros((128, n_sc), np.float32)
        flat = np.arange(S_C) < k_b
        m01[:, :] = flat.reshape(n_sc, 128).T
        in_maps.append(
            dict(
                ctxb=ctxb,
                ctx8=ctx8,
                xr=xr,
                m01=m01,
                wqr=wqr,
                wkvk8=wkvk8,
                wkvv=wkvv,
                wo64=wo64,
                bo_r=bo_r,
                id128=id128,
            )
        )
    return in_maps, n_sc


def _run(inputs, trace=False, **kw):
    in_maps, n_sc = _prep_in_maps(**inputs)
    nc = _get_nc(n_sc)
    res = bass_utils.run_bass_kernel_spmd(
        nc, in_maps, core_ids=list(range(NC_CORES)), trace=trace, **kw
    )
    out = np.stack(
        [res.results[b]["outT"].reshape(E, T).T for b in range(B)]
    ).astype(np.float32)
    return out, res


def kernel(**inputs):
    out, _ = _run(inputs, trace=False)
    return out


if __name__ == "__main__":
    rng = np.random.default_rng(0)
    ins = dict(
        x=rng.standard_normal((B, T, E), dtype=np.float32),
        context=rng.standard_normal((B, S, KV), dtype=np.float32),
        key_padding_mask=rng.integers(0, 2, (B, S)).astype(bool),
        Wq=(rng.standard_normal((512, E), dtype=np.float32) * 0.02),
        Wkv=(rng.standard_normal((1024, KV), dtype=np.float32) * 0.02),
        Wo=(rng.standard_normal((E, 512), dtype=np.float32) * 0.02),
        bo=np.zeros(E, dtype=np.float32),
    )
    out = kernel(**ins)
    print("out", out.shape, out.dtype, np.abs(out).mean())


# revision 42
# speedup vs baseline: 1.0077x; 1.0077x over previous
"""CrossAttention Trainium2 kernel, v2.

Problem (hardcoded): B=8, T=256, S=4096, E=512, KV=768, H=8, D=64.
Sharding: data-parallel over B — one batch per NeuronCore (8 cores).

Key ideas vs v1:
  * Key compaction: ~50% of keys are masked. Host gathers kept keys per
    batch and pads to S_C = ceil(max_kept/128)*128 (2176 for the bench
    seed). All S-proportional work (KV proj, scores, exp, PV) shrinks
    by ~47%. Pad rows have zero context columns -> k=0 -> score 0 ->
    exp 1, but vp rows and the ones-column are 0 so they contribute
    nothing to numerator or denominator (m01 carries kept/pad only).
  * K-projection runs in fp8-e4m3 DoubleRow (contraction 256/pass):
    Wk is rescaled x64 into e4m3's normal range and 2^-6 is folded into
    Wq. Measured end-to-end rel err 1.37e-2 (gate 2e-2), 10us faster.
  * Group-pipelined emission: ctx arrives in groups of 4 s-chunks
    (tapered to 2,2,1 at the end); K/V projection of group g is
    interleaved instruction-by-instruction with attention of group g-1
    so the PE never waits on the scalar-engine exp.
  * Scores keep PE row-group-0 (head-even) and row-group-64 (head-odd)
    outputs in SEPARATE psum banks — mixing them in one bank makes the
    runtime reject the NEFF (PE quadrants are wired to PSUM halves).
  * PV accumulates per group in [65,256] psum tiles (row 64 = softmax
    denominator via a ones-column in vp), folded into SBUF accumulators
    by the DVE.
  * Per-head-pair normalization starts as soon as that pair's last fold
    lands: denominator row is DMA-spread across 128 partitions for a
    fast reciprocal, gathered back, broadcast down 64 rows with a K=1
    matmul; tiny keepalive matmuls tied to the chain keep the PE's HAM
    activity window warm. The out projection contracts in 64-row chunks
    so each head pair feeds it immediately.
"""

import os
import sys

sys.path.insert(0, "/opt/trn_rl_repo")

# Debug truncation: 1=proj, 2=+scores, 3=+exp, 4=+pv, 5=+norm, 6=full (default)
V2_STAGE = int(os.environ.get("V2_STAGE", "6"))
V2_NOILV = os.environ.get("V2_NOILV", "") == "1"  # emit proj before attn (no interleave)

import numpy as np
import ml_dtypes
from contextlib import ExitStack

import concourse.bass as bass
import concourse.bacc as bacc
import concourse.tile as tile
from concourse import mybir
from concourse import bass_utils

BF16 = mybir.dt.bfloat16
F32 = mybir.dt.float32
F8E4 = mybir.dt.float8e4
NPBF16 = ml_dtypes.bfloat16
NPF8 = ml_dtypes.float8_e4m3fn

B, T, S, E, KV, H, D = 8, 256, 4096, 512, 768, 8, 64
NC_CORES = 8
GROUP_SC = 4  # s-chunks per pipeline group


def _build_program(n_sc):
    S_C = n_sc * 128
    sizes = []
    rem = n_sc
    if rem > 7:
        sizes.append(2)  # small first group: projections start sooner
        rem -= 2
    while rem > 5:
        sizes.append(GROUP_SC)
        rem -= GROUP_SC
    while rem > 0:
        n = 2 if rem > 2 else rem
        sizes.append(n)
        rem -= n
    groups = []
    sc0 = 0
    for n in sizes:
        groups.append((sc0, n))
        sc0 += n
    G = len(groups)

    nc = bacc.Bacc("TRN2", target_bir_lowering=False, debug=False)

    ctxb_d = nc.dram_tensor("ctxb", [128, 6 * S_C], BF16, kind="ExternalInput").ap()
    ctx8_d = nc.dram_tensor("ctx8", [128, 6 * S_C], F8E4, kind="ExternalInput").ap()
    wkvk8_d = nc.dram_tensor("wkvk8", [128, 3 * 1024], F8E4, kind="ExternalInput").ap()
    x_d = nc.dram_tensor("xr", [128, 4 * T], BF16, kind="ExternalInput").ap()
    m01_d = nc.dram_tensor("m01", [128, n_sc], F32, kind="ExternalInput").ap()
    wq_d = nc.dram_tensor("wqr", [128, 4 * 512], BF16, kind="ExternalInput").ap()
    wkvv_d = nc.dram_tensor("wkvv", [128, 6 * 512], BF16, kind="ExternalInput").ap()
    wo_d = nc.dram_tensor("wo64", [64, 8 * 512], BF16, kind="ExternalInput").ap()
    bo_d = nc.dram_tensor("bo_r", [128, 4], F32, kind="ExternalInput").ap()
    id128_d = nc.dram_tensor("id128", [128, 128], BF16, kind="ExternalInput").ap()
    outT_d = nc.dram_tensor("outT", [4, 128, T], F32, kind="ExternalOutput").ap()

    with tile.TileContext(nc) as tc, ExitStack() as ctx:
        const = ctx.enter_context(tc.tile_pool(name="const", bufs=1))
        work = ctx.enter_context(tc.tile_pool(name="work", bufs=2))
        psum = ctx.enter_context(tc.tile_pool(name="psum", bufs=1, space="PSUM"))

        # ---- static SBUF tensors ------------------------------------------
        ctxb_t = const.tile([128, 6 * S_C], BF16, tag="ctxb", name="ctxb")
        ctx8_t = const.tile([128, 6 * S_C], F8E4, tag="ctx8", name="ctx8")
        wkvk8_t = const.tile([128, 3 * 1024], F8E4, tag="wkvk8", name="wkvk8")
        x_t = const.tile([128, 4 * T], BF16, tag="x", name="x")
        wq_t = const.tile([128, 4 * 512], BF16, tag="wq", name="wq")
        wkvv_t = const.tile([128, 6 * 512], BF16, tag="wkvv", name="wkvv")
        wo_t = const.tile([64, 8 * 512], BF16, tag="wo", name="wo")
        bo_t = const.tile([128, 4], F32, tag="bo", name="bo")
        m01_t = const.tile([128, n_sc], F32, tag="m01", name="m01")
        kt_t = [
            const.tile([128, S_C], BF16, tag=f"kt{kc}", name=f"kt{kc}")
            for kc in range(4)
        ]
        vp_t = [
            const.tile([128, 8 * 65], BF16, tag=f"vp{sc}", name=f"vp{sc}")
            for sc in range(n_sc)
        ]
        qt_t = [
            const.tile([128, T], BF16, tag=f"qt{qc}", name=f"qt{qc}") for qc in range(4)
        ]
        otE_t = [
            const.tile([64, T], BF16, tag=f"otE{kc}", name=f"otE{kc}")
            for kc in range(4)
        ]
        otO_t = [
            const.tile([64, T], BF16, tag=f"otO{kc}", name=f"otO{kc}")
            for kc in range(4)
        ]
        ones8_t = const.tile([128, 8], BF16, tag="ones8", name="ones8")
        ones64_t = const.tile([1, 64], BF16, tag="ones64", name="ones64")
        id65f_t = const.tile([65, 1], F32, tag="id65f", name="id65f")
        id128_t = const.tile([128, 128], BF16, tag="id128", name="id128")
        recb_t = const.tile([128, 16], BF16, tag="recb", name="recb")
        rechp_t = const.tile([1, 4 * 512], BF16, tag="rechp", name="rechp")
        dummy_t = const.tile([1, 64], BF16, tag="dummy", name="dummy")

        # SBUF accumulators for PV (head pair packed along free; row 64 = denom)
        pvacc_t = [
            const.tile([65, 512], F32, tag=f"pvacc{kc}", name=f"pvacc{kc}")
            for kc in range(4)
        ]

        # ---- tiny init + ACT table warm -----------------------------------
        nc.vector.memset(ones8_t[:], 1.0)
        nc.vector.memset(id65f_t[:], 1.0)
        nc.vector.memset(ones64_t[:], 1.0)
        nc.scalar.activation(
            dummy_t[:], ones64_t[:], mybir.ActivationFunctionType.Exp
        )

        # ---- DMA dispatches (two hw queues; priority order) ---------------
        nc.sync.dma_start(x_t[:], x_d)
        nc.gpsimd.dma_start(wq_t[:], wq_d)
        nc.sync.dma_start(wkvk8_t[:], wkvk8_d)
        ctxb_v = ctxb_t[:].rearrange("p (c s) -> p c s", c=6)
        ctxd_v = ctxb_d.rearrange("p (c s) -> p c s", c=6)
        ctx8_v = ctx8_t[:].rearrange("p (c s) -> p c s", c=6)
        ctx8d_v = ctx8_d.rearrange("p (c s) -> p c s", c=6)
        wkvv_v = wkvv_t[:].rearrange("p (c m) -> p c m", c=6)
        wkvvd_v = wkvv_d.rearrange("p (c m) -> p c m", c=6)
        g0_, gn_ = groups[0]
        cols0 = slice(g0_ * 128, g0_ * 128 + gn_ * 128)
        nc.sync.dma_start(ctx8_v[:, :, cols0], ctx8d_v[:, :, cols0])
        # interleave wkvv and ctx-bf16 group-0 chunks so V-proj can start
        # accumulating as soon as the first c-chunks land
        for c in range(6):
            q = nc.sync if c % 2 == 0 else nc.gpsimd
            q.dma_start(wkvv_v[:, c, :], wkvvd_v[:, c, :])
            q2 = nc.gpsimd if c % 2 == 0 else nc.sync
            q2.dma_start(ctxb_v[:, c, cols0], ctxd_v[:, c, cols0])
        nc.gpsimd.dma_start(m01_t[:], m01_d)
        for gi, (g0, gn) in enumerate(groups[1:], 1):
            cols = slice(g0 * 128, g0 * 128 + gn * 128)
            q = nc.sync if gi % 2 == 0 else nc.gpsimd
            q.dma_start(ctx8_v[:, :, cols], ctx8d_v[:, :, cols])
            for c in range(6):
                q.dma_start(ctxb_v[:, c, cols], ctxd_v[:, c, cols])
        nc.gpsimd.dma_start(wo_t[:], wo_d)
        nc.gpsimd.dma_start(bo_t[:], bo_d)
        nc.gpsimd.dma_start(id128_t[:], id128_d)

        def emit_qproj():
            for qc in range(4):
                ps = psum.tile([128, T], F32, tag="w", bufs=2, name="qps")
                for ec in range(4):
                    nc.tensor.matmul(
                        ps[:],
                        lhsT=wq_t[
                            :, ec * 512 + qc * 128 : ec * 512 + (qc + 1) * 128
                        ],
                        rhs=x_t[:, ec * T : (ec + 1) * T],
                        start=(ec == 0),
                        stop=(ec == 3),
                    )
                nc.scalar.copy(qt_t[qc][:], ps[:])

        # ---- pipeline: proj(g) interleaved with attn(g-1) -----------------
        def proj_ops(gi):
            g0, gn = groups[gi]
            cols = slice(g0 * 128, g0 * 128 + gn * 128)
            ops = []
            for kc in range(4):

                def k_proj(kc=kc, cols=cols, gn=gn, g0=g0, gi=gi):
                    ps = psum.tile([128, 512], F32, tag="w", bufs=2, name="kps")
                    w8 = wkvk8_t[:].rearrange("p (d j m) -> p d j m", d=3, j=2)
                    for dc in range(3):
                        nc.tensor.matmul(
                            ps[:, 0 : gn * 128],
                            lhsT=w8[:, dc, :, kc * 128 : (kc + 1) * 128],
                            rhs=ctx8_v[:, 2 * dc : 2 * dc + 2, cols],
                            start=(dc == 0),
                            stop=(dc == 2),
                            perf_mode=mybir.MatmulPerfMode.DoubleRow,
                        )
                    if gi % 2 == 0:
                        nc.scalar.copy(kt_t[kc][:, cols], ps[:, 0 : gn * 128])
                    else:
                        nc.vector.tensor_copy(kt_t[kc][:, cols], ps[:, 0 : gn * 128])

                ops.append(k_proj)
            for sc in range(g0, g0 + gn):

                def v_proj(sc=sc):
                    ps = psum.tile([128, 512], F32, tag="w", bufs=2, name="vps")
                    for c in range(6):
                        nc.tensor.matmul(
                            ps[:],
                            lhsT=ctxb_v[:, c, sc * 128 : (sc + 1) * 128],
                            rhs=wkvv_t[:, c * 512 : (c + 1) * 512],
                            start=(c == 0),
                            stop=(c == 5),
                        )
                    dst = vp_t[sc][:].rearrange("p (h e) -> p h e", e=65)
                    nc.vector.tensor_copy(
                        dst[:, :, 0:64], ps[:].rearrange("p (h d) -> p h d", d=64)
                    )
                    nc.vector.tensor_scalar_mul(
                        dst[:, :, 64:65],
                        ones8_t[:].rearrange("p (h o) -> p h o", o=1),
                        m01_t[:, sc : sc + 1],
                    )

                ops.append(v_proj)
            return ops

        def attn_units(gi):
            g0, gn = groups[gi]
            pairs = []
            p0 = g0
            while p0 < g0 + gn:
                np_ = min(2, g0 + gn - p0)
                pairs.append((p0, np_))
                p0 += np_
            return [(kc, p0, np_) for kc in range(4) for (p0, np_) in pairs]

        pend_pv = []  # [(kc, p0, np_, eE, eO)]
        n_done = [0] * 4  # PV s-chunks accumulated per kc
        pvq = [None] * 4  # current per-group psum pair for each kc
        fold_done = [False] * 4  # pvacc initialized yet?

        def sc_group(sc):
            for g0, gn in groups:
                if g0 <= sc < g0 + gn:
                    return g0, gn
            raise AssertionError

        def emit_scores(kc, p0, np_):
            if V2_STAGE < 2:
                return
            # separate psum tiles per head: row-group-0 and row-group-64
            # outputs must not share a psum bank
            psE = psum.tile([128, 512], F32, tag="sc", bufs=2, name="psE")
            psO = psum.tile([128, 512], F32, tag="sc", bufs=2, name="psO")
            for i in range(np_):
                sc = p0 + i
                nc.tensor.matmul(
                    psE[:, i * T : (i + 1) * T],
                    lhsT=kt_t[kc][0:64, sc * 128 : (sc + 1) * 128],
                    rhs=qt_t[kc][0:64, :],
                    start=True,
                    stop=True,
                )
            for i in range(np_):
                sc = p0 + i
                nc.tensor.matmul(
                    psO[:, i * T : (i + 1) * T],
                    lhsT=kt_t[kc][64:128, sc * 128 : (sc + 1) * 128],
                    rhs=qt_t[kc][64:128, :],
                    start=True,
                    stop=True,
                )
            if V2_STAGE < 3:
                return
            eE = work.tile([128, 512], BF16, tag="e", bufs=6, name="eE")
            eO = work.tile([128, 512], BF16, tag="e", bufs=6, name="eO")
            nc.scalar.activation(
                eE[:, 0 : np_ * T], psE[:, 0 : np_ * T],
                mybir.ActivationFunctionType.Exp,
            )
            nc.scalar.activation(
                eO[:, 0 : np_ * T], psO[:, 0 : np_ * T],
                mybir.ActivationFunctionType.Exp,
            )
            if V2_STAGE < 4:
                return
            pend_pv.append((kc, p0, np_, eE, eO))

        def emit_pv():
            kc, p0, np_, eE, eO = pend_pv.pop(0)
            g0, gn = sc_group(p0)
            if p0 == g0:
                pvq[kc] = [
                    psum.tile([65, T], F32, tag="pv", bufs=4, name=f"pvq{hi}")
                    for hi in range(2)
                ]
            for i in range(np_):
                sc = p0 + i
                for hi, e in ((0, eE), (1, eO)):
                    h = 2 * kc + hi
                    nc.tensor.matmul(
                        pvq[kc][hi][:],
                        lhsT=vp_t[sc][:, h * 65 : h * 65 + 65],
                        rhs=e[:, i * T : (i + 1) * T],
                        start=(sc == g0),
                        stop=(sc == g0 + gn - 1),
                    )
            if p0 + np_ == g0 + gn:
                # fold the group's PV into the SBUF accumulator (DVE);
                # first fold in processing order initializes it
                for hi in range(2):
                    dst = pvacc_t[kc][:, hi * T : (hi + 1) * T]
                    if not fold_done[kc]:
                        nc.vector.tensor_copy(dst, pvq[kc][hi][:])
                    else:
                        nc.vector.tensor_add(dst, dst, pvq[kc][hi][:])
                fold_done[kc] = True
            n_done[kc] += np_
            if n_done[kc] == n_sc and V2_STAGE >= 5:
                emit_norm(kc)

        def emit_norm(kc):
            # denominators live in pvacc row 64 (cols 0:256 headE, 256:512 headO).
            # Spread the 512 values across 128 partitions (t = p*4+j) so the
            # reciprocal uses 128 DVE lanes instead of 1, then gather back.
            # spread den across 128 partitions via PE transpose (no DMA hop:
            # engine-semaphore sync only, and the PE stays active)
            dsp = psum.tile([128, 512], F32, tag="sc", bufs=2, name="dsp")
            for c in range(4):
                nc.tensor.matmul(
                    dsp[:, c : c + 1],
                    lhsT=pvacc_t[kc][64:65, c * 128 : (c + 1) * 128],
                    rhs=id65f_t[64:65, :],
                    start=True,
                    stop=True,
                    is_transpose=True,
                )
            with nc.allow_low_precision(reason="softmax denom reciprocal in bf16"):
                nc.vector.reciprocal(
                    recb_t[:, kc * 4 : (kc + 1) * 4], dsp[:, 0:4]
                )
            # gather back to one [1,512] row via 4 transposes, then to SBUF
            rgp = psum.tile([1, 512], BF16, tag="sc", bufs=2, name="rgp")
            for j in range(4):
                nc.tensor.matmul(
                    rgp[0:1, j * 128 : (j + 1) * 128],
                    lhsT=recb_t[:, kc * 4 + j : kc * 4 + j + 1],
                    rhs=id128_t[:],
                    start=True,
                    stop=True,
                    is_transpose=True,
                )
            nc.vector.tensor_copy(
                rechp_t[0:1, kc * 512 : (kc + 1) * 512], rgp[0:1, :]
            )
            bc = psum.tile([128, 512], F32, tag="sc", bufs=2, name="bc")
            nc.tensor.matmul(
                bc[0:64, :],
                lhsT=ones64_t[:],
                rhs=rechp_t[0:1, kc * 512 : (kc + 1) * 512],
                start=True,
                stop=True,
            )
            nc.vector.tensor_mul(otE_t[kc][:], pvacc_t[kc][0:64, 0:T], bc[0:64, 0:T])
            nc.vector.tensor_mul(
                otO_t[kc][:], pvacc_t[kc][0:64, T : 2 * T], bc[0:64, T : 2 * T]
            )
            if V2_STAGE < 6:
                return
            # out projection contribution of this head pair (64-row chunks)
            for j, ot in ((2 * kc, otE_t[kc]), (2 * kc + 1, otO_t[kc])):
                for oi in range(4):
                    nc.tensor.matmul(
                        out_ps[oi // 2][:, (oi % 2) * T : (oi % 2 + 1) * T],
                        lhsT=wo_t[:, j * 512 + oi * 128 : j * 512 + (oi + 1) * 128],
                        rhs=ot[:],
                        start=(j == 0 and oi % 2 == 0),
                        stop=(j == 7 and oi % 2 == 1),
                    )

        # Reserve early-group attention units per kc and drain them in the
        # final phase: staggers the four pv completions so each norm chain
        # (den DMA -> recip -> gather -> bc -> muls -> outproj) overlaps real
        # PE/ACT work instead of stalling the PE (and re-throttling HAM).
        reserved = {kc: set() for kc in range(4)}

        def group_index(g0):
            for idx, (gg0, _) in enumerate(groups):
                if gg0 == g0:
                    return idx
            raise AssertionError

        out_ps = None
        emit_qproj()
        for gi in range(G + 1):
            units = attn_units(gi - 1) if gi >= 1 else []
            units = [
                (kc, p0, np_)
                for (kc, p0, np_) in units
                if group_index(sc_group(p0)[0]) not in reserved[kc]
            ]
            pops = proj_ops(gi) if gi < G else []
            if gi == G:
                # final phase: append reserved units ordered so each kc's
                # completion lands a few units after the previous kc's norm
                for kc in range(1, 4):
                    for g_idx in sorted(reserved[kc]):
                        g0r, gnr = groups[g_idx]
                        p0 = g0r
                        while p0 < g0r + gnr:
                            np_ = min(2, g0r + gnr - p0)
                            units.append((kc, p0, np_))
                            p0 += np_
                # allocate out-projection accumulators (w-ring is free now)
                out_ps = [
                    psum.tile([128, 512], F32, tag="w", bufs=2, name=f"ops{i}")
                    for i in range(2)
                ]
            if V2_STAGE < 2:
                units = []
            if not units:
                for op in pops:
                    op()
                continue
            # interleave proj ops between attention units
            if V2_NOILV:
                for op in pops:
                    op()
                pops = []
            k = 0
            for i, (kc, p0, np_) in enumerate(units):
                emit_scores(kc, p0, np_)
                while len(pend_pv) > 1:
                    emit_pv()
                k_to = (i + 1) * len(pops) // len(units) if pops else 0
                while k < k_to:
                    pops[k]()
                    k += 1
        while pend_pv:
            emit_pv()

        # ---- bias + store -------------------------------------------------
        for half in range(2):
            osb = work.tile([128, 512], F32, tag="osb", bufs=2, name="osb")
            if V2_STAGE >= 6:
                for eo2 in range(2):
                    eo = half * 2 + eo2
                    nc.vector.tensor_scalar_add(
                        osb[:, eo2 * T : (eo2 + 1) * T],
                        out_ps[half][:, eo2 * T : (eo2 + 1) * T],
                        bo_t[:, eo : eo + 1],
                    )
            else:
                nc.vector.memset(osb[:], 0.0)
            for eo2 in range(2):
                q = nc.sync if eo2 == 0 else nc.gpsimd
                q.dma_start(
                    outT_d[2 * half + eo2], osb[:, eo2 * T : (eo2 + 1) * T]
                )

    nc.compile()
    return nc


_NC = {}


def _get_nc(n_sc):
    if n_sc not in _NC:
        _NC[n_sc] = _build_program(n_sc)
    return _NC[n_sc]


def _prep_in_maps(x, context, key_padding_mask, Wq, Wkv, Wo, bo):
    keep = ~np.asarray(key_padding_mask)
    kept = keep.sum(axis=1)
    n_sc = max(1, -(-int(kept.max()) // 128))
    S_C = n_sc * 128

    scale = np.float32(D**-0.5) * np.float32(2.0**-6)
    wqr = (
        (np.ascontiguousarray(Wq.T) * scale)
        .reshape(4, 128, 4 * 128)
        .transpose(1, 0, 2)
        .reshape(128, 4 * 512)
        .astype(NPBF16)
    )
    wkvT = np.ascontiguousarray(Wkv.T)  # [768, 1024]
    wkvk8 = (
        (wkvT[:, :512] * np.float32(64.0))
        .astype(NPF8)
        .reshape(3, 2, 128, 512)
        .transpose(2, 0, 1, 3)
        .reshape(128, 3 * 1024)
    )
    wkvv = (
        wkvT[:, 512:].reshape(6, 128, 512).transpose(1, 0, 2).reshape(128, 6 * 512)
    ).astype(NPBF16)
    wo64 = (
        np.ascontiguousarray(Wo.T)
        .reshape(8, 64, 512)
        .transpose(1, 0, 2)
        .reshape(64, 8 * 512)
    ).astype(NPBF16)
    bo_r = np.ascontiguousarray(bo.reshape(4, 128).T).astype(np.float32)
    id128 = np.eye(128, dtype=np.float32).astype(NPBF16)

    in_maps = []
    for b in range(B):
        idx = np.nonzero(keep[b])[0]
        k_b = len(idx)
        ctxT = np.zeros((KV, S_C), np.float32)
        ctxT[:, :k_b] = context[b][idx].T
        ctxr = ctxT.reshape(6, 128, S_C).transpose(1, 0, 2).reshape(128, 6 * S_C)
        ctxb = ctxr.astype(NPBF16)
        ctx8 = ctxr.astype(NPF8)
        xr = (
            np.ascontiguousarray(x[b].T)
            .reshape(4, 128, T)
            .transpose(1, 0, 2)
            .reshape(128, 4 * T)
        ).astype(NPBF16)
        m01 = np.zeros((128, n_sc), np.float32)
        flat = np.arange(S_C) < k_b
        m01[:, :] = flat.reshape(n_sc, 128).T
        in_maps.append(
            dict(
                ctxb=ctxb,
                ctx8=ctx8,
                xr=xr,
                m01=m01,
                wqr=wqr,
                wkvk8=wkvk8,
                wkvv=wkvv,
                wo64=wo64,
                bo_r=bo_r,
                id128=id128,
            )
        )
    return in_maps, n_sc


def _run(inputs, trace=False, **kw):
    in_maps, n_sc = _prep_in_maps(**inputs)
    nc = _get_nc(n_sc)
    res = bass_utils.run_bass_kernel_spmd(
        nc, in_maps, core_ids=list(range(NC_CORES)), trace=trace, **kw
    )
    out = np.stack(
        [res.results[b]["outT"].reshape(E, T).T for b in range(B)]
    ).astype(np.float32)
    return out, res


def kernel(**inputs):
    out, _ = _run(inputs, trace=False)
    return out


if __name__ == "__main__":
    rng = np.random.default_rng(0)
    ins = dict(
        x=rng.standard_normal((B, T, E), dtype=np.float32),
        context=rng.standard_normal((B, S, KV), dtype=np.float32),
        key_padding_mask=rng.integers(0, 2, (B, S)).astype(bool),
        Wq=(rng.standard_normal((512, E), dtype=np.float32) * 0.02),
        Wkv=(rng.standard_normal((1024, KV), dtype=np.float32) * 0.02),
        Wo=(rng.standard_normal((E, 512), dtype=np.float32) * 0.02),
        bo=np.zeros(E, dtype=np.float32),
    )
    out = kernel(**ins)
    print("out", out.shape, out.dtype, np.abs(out).mean())
